# revision 16
# baseline (speedup 1.0000x reference)
"""7x7 valid conv2d (cross-correlation) on a 4096x4096 fp32 image, 8 NeuronCores.

Strategy: 2x4 core grid (2 row bands x 4 col bands), halo baked in on the host
so there are no device collectives.  Per core the conv runs on the TensorEngine
as 7 PSUM-accumulated "banded Toeplitz" matmuls per (row-stripe, col-tile):
for each kernel column kx, a [K=128, M=122] stationary matrix T_kx with
T_kx[m+ky, m] = w[ky, kx] contracts 128 input rows into 122 output rows; the
kx shift is a free column offset on the moving operand.

v2 changes vs the 76.1us baseline (trace-driven):
- warm stream was already perfect (216.4 ns/MM = N/2.4+2.5, zero bubbles), so
  all wins are overhead: startup and the store tail.
- dropped the 8 dummy warmup matmuls + memset: they occupied the PE 8.2->11.6us
  while the real stream was ready at 8.2us; letting the real MMs run the HAM
  cold window does the same warmup with useful work.
- first DMAs reordered/split (T_0 chunk, then x cols :518) so the first real
  matmul issues as soon as the sync queue drains the preamble.
- output staging is one wide SBUF tile [128, 17*1023]; DRAM out is
  o_d[p, s*1023+c] = out[122*s + p, c] so each partition's stripes are
  CONTIGUOUS in DRAM.  Stores ship in 5 grouped SWDGE dma_starts with
  multi-KB per-partition descriptors instead of 20 stores of 2KB rows
  (the old store path ran at 79 GB/s descriptor-bound and left a 7.8us
  post-compute flush).
"""

import numpy as np
import ml_dtypes

import concourse.bacc as bacc
import concourse.bass as bass
import concourse.tile as tile
import concourse.mybir as mybir
from concourse.bass_utils import run_bass_kernel_spmd

H = W = 4096
KH = KW = 7
OH = OW = H - KH + 1  # 4090
NCORES = 8
GR, GC = 2, 4                  # core grid: 2 row bands x 4 col bands
ROWS_PC = 2045                 # output rows per core
COLS_PC = 1023                 # output cols per core (col bands overlap by 2)
ROW_BAND = [0, 2045]
COL_BAND = [0, 1023, 2046, 3067]   # last band overlaps band 2 by 2 cols
MT = 122                       # output rows per stripe (contraction K = 128)
ROW_STARTS = list(range(0, ROWS_PC, MT))  # 17 stripes, last keeps 93 rows
NSTRIPES = len(ROW_STARTS)                # 17
IN_ROWS = ROW_STARTS[-1] + 128            # 2080 (2051 real + pad)
IN_COLS = 1032                            # 1023 + 6 halo + 3 pad
COL_TILES = [(0, 512), (512, 511)]        # (c0, N) psum col tiles

MODE = "bf16"
TRACE = False
LAST_EXEC_NS = None
# Stores must use SWDGE (gpsimd): HWDGE stores of this pattern measured
# ~25 GB/s (120us total).  Each SWDGE dma_start lands on ONE SDMA engine
# pair (~50 GB/s), pairs rotating per dma_start - so ship early/small and
# split the late stores to engage multiple pairs in parallel.

_DT = {
    "bf16": (mybir.dt.bfloat16, ml_dtypes.bfloat16),
    "fp32": (mybir.dt.float32, np.float32),
}

_compiled = {}


def _build(mode):
    dt_b, _ = _DT[mode]
    nc = bacc.Bacc(
        "TRN2", target_bir_lowering=False, debug=False, num_devices=NCORES
    )
    x_d = nc.dram_tensor("x", [IN_ROWS, IN_COLS], dt_b, kind="ExternalInput").ap()
    t_d = nc.dram_tensor("tmats", [128, KW * MT], dt_b, kind="ExternalInput").ap()
    # bf16 output staging/store halves store traffic; host upcasts.
    # layout: o_d[p, s*COLS_PC + c] = out[122*s + p, c]  (stripe-contiguous
    # per partition -> multi-KB store descriptors)
    o_d = nc.dram_tensor(
        "out", [128, NSTRIPES * COLS_PC], mybir.dt.bfloat16, kind="ExternalOutput"
    ).ap()

    with tile.TileContext(nc) as tc:
        with (
            tc.tile_pool(name="tmat", bufs=1) as tpool,
            tc.tile_pool(name="xsl", bufs=7) as xpool,
            tc.tile_pool(name="ps", bufs=7, space="PSUM") as ppool,
            tc.tile_pool(name="ost", bufs=1) as opool,
        ):
            tm = tpool.tile([128, KW * MT], dt_b)
            # one wide staging tile; stripe s casts into cols [s*1023,(s+1)*1023)
            ot = opool.tile([128, NSTRIPES * COLS_PC], mybir.dt.bfloat16)
            for ti, r0 in enumerate(ROW_STARTS):
                M = min(MT, ROWS_PC - r0)  # rows actually kept
                xt = xpool.tile([128, IN_COLS], dt_b, tag="x")
                if ti == 0:
                    # gate the first matmuls on as little DMA as possible:
                    # T_0..T_2, then moving cols 0:518, then the rest
                    nc.sync.dma_start(tm[:, : 3 * MT], t_d[:, : 3 * MT])
                    nc.sync.dma_start(xt[:, :518], x_d[r0 : r0 + 128, :518])
                    nc.sync.dma_start(tm[:, 3 * MT :], t_d[:, 3 * MT :])
                    nc.sync.dma_start(xt[:, 518:], x_d[r0 : r0 + 128, 518:])
                else:
                    nc.sync.dma_start(xt[:, :], x_d[r0 : r0 + 128, :])
                oc = ti * COLS_PC
                for ci, (c0, N) in enumerate(COL_TILES):
                    ps = ppool.tile([MT, 512], mybir.dt.float32, tag="ps")
                    for kx in range(KW):
                        nc.tensor.matmul(
                            ps[:, :N],
                            tm[:, kx * MT : kx * MT + MT],
                            xt[:, c0 + kx : c0 + kx + N],
                            start=(kx == 0),
                            stop=(kx == KW - 1),
                        )
                    # PSUM drain split across DVE and ACT so the two col-tiles'
                    # casts run in parallel (shortens the end-of-stream drain)
                    if ti == NSTRIPES - 1 and ci == 1:
                        # final cast is on the critical path: split it across
                        # both engines (32-aligned partition split)
                        h2 = 64
                        nc.vector.tensor_copy(
                            ot[:h2, oc + c0 : oc + c0 + N], ps[:h2, :N]
                        )
                        nc.scalar.copy(
                            ot[h2:M, oc + c0 : oc + c0 + N], ps[h2:M, :N]
                        )
                    elif ci == 0:
                        nc.vector.tensor_copy(ot[:M, oc + c0 : oc + c0 + N], ps[:M, :N])
                    else:
                        nc.scalar.copy(ot[:M, oc + c0 : oc + c0 + N], ps[:M, :N])
                if ti < NSTRIPES - 1:
                    nc.gpsimd.dma_start(
                        o_d[:M, oc : oc + COLS_PC], ot[:M, oc : oc + COLS_PC]
                    )
                else:
                    # tail: full-width rows only (2046B descriptors - col
                    # slices would fragment them), split by partition half
                    # across BOTH DGE paths so desc-gen and flush run in
                    # parallel on independent rings (32-aligned split)
                    h = 48
                    nc.sync.dma_start(
                        o_d[:h, oc : oc + COLS_PC], ot[:h, oc : oc + COLS_PC]
                    )
                    nc.gpsimd.dma_start(
                        o_d[h:M, oc : oc + COLS_PC], ot[h:M, oc : oc + COLS_PC]
                    )
    nc.compile()
    return nc


def _toeplitz(weight, np_dt):
    t = np.zeros((128, KW * MT), dtype=np.float32)
    idx = np.arange(MT)
    for kx in range(KW):
        for ky in range(KH):
            t[idx + ky, kx * MT + idx] = weight[ky, kx]
    return np.ascontiguousarray(t.astype(np_dt))


def kernel(x, weight):
    global LAST_EXEC_NS
    mode = MODE
    dt_b, np_dt = _DT[mode]
    if mode not in _compiled:
        _compiled[mode] = _build(mode)
    nc = _compiled[mode]

    xf = np.asarray(x, np.float32)
    wf = np.asarray(weight, np.float32)
    tmats = _toeplitz(wf, np_dt)
    xc = xf.astype(np_dt) if np_dt is not np.float32 else xf

    # padded canvas so every core's slab is [IN_ROWS, IN_COLS]
    xpad = np.zeros((ROW_BAND[-1] + IN_ROWS, COL_BAND[-1] + IN_COLS), dtype=xc.dtype)
    xpad[:H, :W] = xc
    in_maps = []
    for c in range(NCORES):
        r0, c0 = ROW_BAND[c // GC], COL_BAND[c % GC]
        in_maps.append(
            {
                "x": np.ascontiguousarray(xpad[r0 : r0 + IN_ROWS, c0 : c0 + IN_COLS]),
                "tmats": tmats,
            }
        )
    res = run_bass_kernel_spmd(
        nc, in_maps, core_ids=list(range(NCORES)), trace=TRACE
    )
    LAST_EXEC_NS = res.exec_time_ns

    out = np.empty((OH, OW), np.float32)
    for c in range(NCORES):
        r0, c0 = ROW_BAND[c // GC], COL_BAND[c % GC]
        od = res.results[c]["out"]  # [128, 17*1023]
        # od[p, s*1023 + cc] = out_core[122*s + p, cc]
        rows = (
            od.reshape(128, NSTRIPES, COLS_PC)
            .swapaxes(0, 1)
            .reshape(NSTRIPES * 128, COLS_PC)
        )
        # stripe s contributes rows 122s..122s+121 -> take p<122 of each stripe
        rows = rows.reshape(NSTRIPES, 128, COLS_PC)[:, :MT, :].reshape(-1, COLS_PC)
        out[r0 : r0 + ROWS_PC, c0 : c0 + COLS_PC] = rows[:ROWS_PC].astype(np.float32)
    return out


# revision 19
# speedup vs baseline: 1.0072x; 1.0072x over previous
"""7x7 valid conv2d (cross-correlation) on a 4096x4096 fp32 image, 8 NeuronCores.

Strategy: 2x4 core grid (2 row bands x 4 col bands), halo baked in on the host
so there are no device collectives.  Per core the conv runs on the TensorEngine
as 7 PSUM-accumulated "banded Toeplitz" matmuls per (row-stripe, col-tile):
for each kernel column kx, a [K=128, M=122] stationary matrix T_kx with
T_kx[m+ky, m] = w[ky, kx] contracts 128 input rows into 122 output rows; the
kx shift is a free column offset on the moving operand.

v2 changes vs the 76.1us baseline (trace-driven):
- warm stream was already perfect (216.4 ns/MM = N/2.4+2.5, zero bubbles), so
  all wins are overhead: startup and the store tail.
- dropped the 8 dummy warmup matmuls + memset: they occupied the PE 8.2->11.6us
  while the real stream was ready at 8.2us; letting the real MMs run the HAM
  cold window does the same warmup with useful work.
- first DMAs reordered/split (T_0 chunk, then x cols :518) so the first real
  matmul issues as soon as the sync queue drains the preamble.
- output staging is one wide SBUF tile [128, 17*1023]; DRAM out is
  o_d[p, s*1023+c] = out[122*s + p, c] so each partition's stripes are
  CONTIGUOUS in DRAM.  Stores ship in 5 grouped SWDGE dma_starts with
  multi-KB per-partition descriptors instead of 20 stores of 2KB rows
  (the old store path ran at 79 GB/s descriptor-bound and left a 7.8us
  post-compute flush).
"""

import numpy as np
import ml_dtypes

import concourse.bacc as bacc
import concourse.bass as bass
import concourse.tile as tile
import concourse.mybir as mybir
from concourse.bass_utils import run_bass_kernel_spmd

H = W = 4096
KH = KW = 7
OH = OW = H - KH + 1  # 4090
NCORES = 8
GR, GC = 2, 4                  # core grid: 2 row bands x 4 col bands
ROWS_PC = 2045                 # output rows per core
COLS_PC = 1023                 # output cols per core (col bands overlap by 2)
ROW_BAND = [0, 2045]
COL_BAND = [0, 1023, 2046, 3067]   # last band overlaps band 2 by 2 cols
MT = 122                       # output rows per stripe (contraction K = 128)
ROW_STARTS = list(range(0, ROWS_PC, MT))  # 17 stripes, last keeps 93 rows
NSTRIPES = len(ROW_STARTS)                # 17
IN_ROWS = ROW_STARTS[-1] + 128            # 2080 (2051 real + pad)
IN_COLS = 1032                            # 1023 + 6 halo + 3 pad
COL_TILES = [(0, 512), (512, 511)]        # (c0, N) psum col tiles

MODE = "bf16"
TRACE = False
LAST_EXEC_NS = None
# Stores must use SWDGE (gpsimd): HWDGE stores of this pattern measured
# ~25 GB/s (120us total).  Each SWDGE dma_start lands on ONE SDMA engine
# pair (~50 GB/s), pairs rotating per dma_start - so ship early/small and
# split the late stores to engage multiple pairs in parallel.

_DT = {
    "bf16": (mybir.dt.bfloat16, ml_dtypes.bfloat16),
    "fp32": (mybir.dt.float32, np.float32),
}

_compiled = {}


def _build(mode):
    dt_b, _ = _DT[mode]
    nc = bacc.Bacc(
        "TRN2", target_bir_lowering=False, debug=False, num_devices=NCORES
    )
    x_d = nc.dram_tensor("x", [IN_ROWS, IN_COLS], dt_b, kind="ExternalInput").ap()
    t_d = nc.dram_tensor("tmats", [128, KW * MT], dt_b, kind="ExternalInput").ap()
    # bf16 output staging/store halves store traffic; host upcasts.
    # layout: o_d[p, s*COLS_PC + c] = out[122*s + p, c]  (stripe-contiguous
    # per partition -> multi-KB store descriptors)
    o_d = nc.dram_tensor(
        "out", [128, NSTRIPES * COLS_PC], mybir.dt.bfloat16, kind="ExternalOutput"
    ).ap()

    with tile.TileContext(nc) as tc:
        with (
            tc.tile_pool(name="tmat", bufs=1) as tpool,
            tc.tile_pool(name="xsl", bufs=7) as xpool,
            tc.tile_pool(name="ps", bufs=7, space="PSUM") as ppool,
            tc.tile_pool(name="ost", bufs=1) as opool,
        ):
            tm = tpool.tile([128, KW * MT], dt_b)
            # one wide staging tile; stripe s casts into cols [s*1023,(s+1)*1023)
            ot = opool.tile([128, NSTRIPES * COLS_PC], mybir.dt.bfloat16)
            for ti, r0 in enumerate(ROW_STARTS):
                M = min(MT, ROWS_PC - r0)  # rows actually kept
                xt = xpool.tile([128, IN_COLS], dt_b, tag="x")
                if ti == 0:
                    # gate the first matmuls on as little DMA as possible:
                    # T_0..T_2, then moving cols 0:518 (in two pipelined
                    # chunks), then the rest
                    nc.sync.dma_start(tm[:, : 3 * MT], t_d[:, : 3 * MT])
                    nc.sync.dma_start(xt[:, :262], x_d[r0 : r0 + 128, :262])
                    nc.sync.dma_start(xt[:, 262:518], x_d[r0 : r0 + 128, 262:518])
                    nc.sync.dma_start(tm[:, 3 * MT :], t_d[:, 3 * MT :])
                    nc.sync.dma_start(xt[:, 518:], x_d[r0 : r0 + 128, 518:])
                else:
                    nc.sync.dma_start(xt[:, :], x_d[r0 : r0 + 128, :])
                oc = ti * COLS_PC
                for ci, (c0, N) in enumerate(COL_TILES):
                    ps = ppool.tile([MT, 512], mybir.dt.float32, tag="ps")
                    for kx in range(KW):
                        nc.tensor.matmul(
                            ps[:, :N],
                            tm[:, kx * MT : kx * MT + MT],
                            xt[:, c0 + kx : c0 + kx + N],
                            start=(kx == 0),
                            stop=(kx == KW - 1),
                        )
                    # PSUM drain split across DVE and ACT so the two col-tiles'
                    # casts run in parallel (shortens the end-of-stream drain)
                    if ti == NSTRIPES - 1 and ci == 1:
                        # final cast on DVE (Scalar dispatches late here)
                        nc.vector.tensor_copy(
                            ot[:M, oc + c0 : oc + c0 + N], ps[:M, :N]
                        )
                    elif ci == 0:
                        nc.vector.tensor_copy(ot[:M, oc + c0 : oc + c0 + N], ps[:M, :N])
                    else:
                        nc.scalar.copy(ot[:M, oc + c0 : oc + c0 + N], ps[:M, :N])
                if ti < NSTRIPES - 1:
                    nc.gpsimd.dma_start(
                        o_d[:M, oc : oc + COLS_PC], ot[:M, oc : oc + COLS_PC]
                    )
                else:
                    # tail: full-width rows only (2046B descriptors - col
                    # slices would fragment them), split by partition half so
                    # the final flush runs on two SWDGE engine pairs in
                    # parallel (HWDGE stores measured 15 GB/s - never use)
                    h = 48
                    nc.gpsimd.dma_start(
                        o_d[:h, oc : oc + COLS_PC], ot[:h, oc : oc + COLS_PC]
                    )
                    nc.gpsimd.dma_start(
                        o_d[h:M, oc : oc + COLS_PC], ot[h:M, oc : oc + COLS_PC]
                    )
    nc.compile()
    return nc


def _toeplitz(weight, np_dt):
    t = np.zeros((128, KW * MT), dtype=np.float32)
    idx = np.arange(MT)
    for kx in range(KW):
        for ky in range(KH):
            t[idx + ky, kx * MT + idx] = weight[ky, kx]
    return np.ascontiguousarray(t.astype(np_dt))


def kernel(x, weight):
    global LAST_EXEC_NS
    mode = MODE
    dt_b, np_dt = _DT[mode]
    if mode not in _compiled:
        _compiled[mode] = _build(mode)
    nc = _compiled[mode]

    xf = np.asarray(x, np.float32)
    wf = np.asarray(weight, np.float32)
    tmats = _toeplitz(wf, np_dt)
    xc = xf.astype(np_dt) if np_dt is not np.float32 else xf

    # padded canvas so every core's slab is [IN_ROWS, IN_COLS]
    xpad = np.zeros((ROW_BAND[-1] + IN_ROWS, COL_BAND[-1] + IN_COLS), dtype=xc.dtype)
    xpad[:H, :W] = xc
    in_maps = []
    for c in range(NCORES):
        r0, c0 = ROW_BAND[c // GC], COL_BAND[c % GC]
        in_maps.append(
            {
                "x": np.ascontiguousarray(xpad[r0 : r0 + IN_ROWS, c0 : c0 + IN_COLS]),
                "tmats": tmats,
            }
        )
    res = run_bass_kernel_spmd(
        nc, in_maps, core_ids=list(range(NCORES)), trace=TRACE
    )
    LAST_EXEC_NS = res.exec_time_ns

    out = np.empty((OH, OW), np.float32)
    for c in range(NCORES):
        r0, c0 = ROW_BAND[c // GC], COL_BAND[c % GC]
        od = res.results[c]["out"]  # [128, 17*1023]
        # od[p, s*1023 + cc] = out_core[122*s + p, cc]
        rows = (
            od.reshape(128, NSTRIPES, COLS_PC)
            .swapaxes(0, 1)
            .reshape(NSTRIPES * 128, COLS_PC)
        )
        # stripe s contributes rows 122s..122s+121 -> take p<122 of each stripe
        rows = rows.reshape(NSTRIPES, 128, COLS_PC)[:, :MT, :].reshape(-1, COLS_PC)
        out[r0 : r0 + ROWS_PC, c0 : c0 + COLS_PC] = rows[:ROWS_PC].astype(np.float32)
    return out


# revision 21
# speedup vs baseline: 1.0254x; 1.0181x over previous
"""7x7 valid conv2d (cross-correlation) on a 4096x4096 fp32 image, 8 NeuronCores.

Strategy: 2x4 core grid (2 row bands x 4 col bands), halo baked in on the host
so there are no device collectives.  Per core the conv runs on the TensorEngine
as 7 PSUM-accumulated "banded Toeplitz" matmuls per (row-stripe, col-tile):
for each kernel column kx, a [K=128, M=122] stationary matrix T_kx with
T_kx[m+ky, m] = w[ky, kx] contracts 128 input rows into 122 output rows; the
kx shift is a free column offset on the moving operand.

v2 changes vs the 76.1us baseline (trace-driven):
- warm stream was already perfect (216.4 ns/MM = N/2.4+2.5, zero bubbles), so
  all wins are overhead: startup and the store tail.
- dropped the 8 dummy warmup matmuls + memset: they occupied the PE 8.2->11.6us
  while the real stream was ready at 8.2us; letting the real MMs run the HAM
  cold window does the same warmup with useful work.
- first DMAs reordered/split (T_0 chunk, then x cols :518) so the first real
  matmul issues as soon as the sync queue drains the preamble.
- output staging is one wide SBUF tile [128, 17*1023]; DRAM out is
  o_d[p, s*1023+c] = out[122*s + p, c] so each partition's stripes are
  CONTIGUOUS in DRAM.  Stores ship in 5 grouped SWDGE dma_starts with
  multi-KB per-partition descriptors instead of 20 stores of 2KB rows
  (the old store path ran at 79 GB/s descriptor-bound and left a 7.8us
  post-compute flush).
"""

import numpy as np
import ml_dtypes

import concourse.bacc as bacc
import concourse.bass as bass
import concourse.tile as tile
import concourse.mybir as mybir
from concourse.bass_utils import run_bass_kernel_spmd

H = W = 4096
KH = KW = 7
OH = OW = H - KH + 1  # 4090
NCORES = 8
GR, GC = 2, 4                  # core grid: 2 row bands x 4 col bands
ROWS_PC = 2045                 # output rows per core
COLS_PC = 1023                 # output cols per core (col bands overlap by 2)
ROW_BAND = [0, 2045]
COL_BAND = [0, 1023, 2046, 3067]   # last band overlaps band 2 by 2 cols
MT = 122                       # output rows per stripe (contraction K = 128)
ROW_STARTS = list(range(0, ROWS_PC, MT))  # 17 stripes, last keeps 93 rows
NSTRIPES = len(ROW_STARTS)                # 17
IN_ROWS = ROW_STARTS[-1] + 128            # 2080 (2051 real + pad)
IN_COLS = 1032                            # 1023 + 6 halo + 3 pad
COL_TILES = [(0, 512), (512, 511)]        # (c0, N) psum col tiles

MODE = "bf16"
TRACE = False
LAST_EXEC_NS = None
# Stores must use SWDGE (gpsimd): HWDGE stores of this pattern measured
# ~25 GB/s (120us total).  Each SWDGE dma_start lands on ONE SDMA engine
# pair (~50 GB/s), pairs rotating per dma_start - so ship early/small and
# split the late stores to engage multiple pairs in parallel.

_DT = {
    "bf16": (mybir.dt.bfloat16, ml_dtypes.bfloat16),
    "fp32": (mybir.dt.float32, np.float32),
}

_compiled = {}


def _build(mode):
    dt_b, _ = _DT[mode]
    nc = bacc.Bacc(
        "TRN2", target_bir_lowering=False, debug=False, num_devices=NCORES
    )
    x_d = nc.dram_tensor("x", [IN_ROWS, IN_COLS], dt_b, kind="ExternalInput").ap()
    t_d = nc.dram_tensor("tmats", [128, KW * MT], dt_b, kind="ExternalInput").ap()
    # bf16 output staging/store halves store traffic; host upcasts.
    # layout: o_d[p, s*COLS_PC + c] = out[122*s + p, c]  (stripe-contiguous
    # per partition -> multi-KB store descriptors)
    o_d = nc.dram_tensor(
        "out", [128, NSTRIPES * COLS_PC], mybir.dt.bfloat16, kind="ExternalOutput"
    ).ap()

    with tile.TileContext(nc) as tc:
        with (
            tc.tile_pool(name="tmat", bufs=1) as tpool,
            tc.tile_pool(name="xsl", bufs=7) as xpool,
            tc.tile_pool(name="ps", bufs=7, space="PSUM") as ppool,
            tc.tile_pool(name="ost", bufs=1) as opool,
        ):
            tm = tpool.tile([128, KW * MT], dt_b)
            # one wide staging tile; stripe s casts into cols [s*1023,(s+1)*1023)
            ot = opool.tile([128, NSTRIPES * COLS_PC], mybir.dt.bfloat16)
            for ti, r0 in enumerate(ROW_STARTS):
                M = min(MT, ROWS_PC - r0)  # rows actually kept
                xt = xpool.tile([128, IN_COLS], dt_b, tag="x")
                if ti == 0:
                    # gate the first matmuls on as little DMA as possible:
                    # T_0..T_2, then moving cols 0:518, then the rest.
                    # (do NOT chunk finer: each HWDGE dma_start has a ~2us
                    # end-to-end latency floor, so extra sems serialize)
                    nc.sync.dma_start(tm[:, : 3 * MT], t_d[:, : 3 * MT])
                    nc.sync.dma_start(xt[:, :518], x_d[r0 : r0 + 128, :518])
                    nc.sync.dma_start(tm[:, 3 * MT :], t_d[:, 3 * MT :])
                    nc.sync.dma_start(xt[:, 518:], x_d[r0 : r0 + 128, 518:])
                else:
                    nc.sync.dma_start(xt[:, :], x_d[r0 : r0 + 128, :])
                oc = ti * COLS_PC
                for ci, (c0, N) in enumerate(COL_TILES):
                    ps = ppool.tile([MT, 512], mybir.dt.float32, tag="ps")
                    for kx in range(KW):
                        nc.tensor.matmul(
                            ps[:, :N],
                            tm[:, kx * MT : kx * MT + MT],
                            xt[:, c0 + kx : c0 + kx + N],
                            start=(kx == 0),
                            stop=(kx == KW - 1),
                        )
                    # PSUM drain split across DVE and ACT so the two col-tiles'
                    # casts run in parallel (shortens the end-of-stream drain)
                    if ti == NSTRIPES - 1 and ci == 1:
                        # final cast on DVE (Scalar dispatches late here)
                        nc.vector.tensor_copy(
                            ot[:M, oc + c0 : oc + c0 + N], ps[:M, :N]
                        )
                    elif ci == 0:
                        nc.vector.tensor_copy(ot[:M, oc + c0 : oc + c0 + N], ps[:M, :N])
                    else:
                        nc.scalar.copy(ot[:M, oc + c0 : oc + c0 + N], ps[:M, :N])
                # Every stripe ships as TWO partition-half stores, full width
                # (2046B descriptors - col slices would fragment them).  Each
                # SWDGE dma_start lands on one SDMA engine pair (~53 GB/s);
                # 125KB halves flush in ~2.4us so the 8 rotating pairs never
                # collide and the final halves always find free pairs.
                # (HWDGE stores measured 15 GB/s - never use for stores.)
                h = 64 if M > 64 else 48
                nc.gpsimd.dma_start(
                    o_d[:h, oc : oc + COLS_PC], ot[:h, oc : oc + COLS_PC]
                )
                nc.gpsimd.dma_start(
                    o_d[h:M, oc : oc + COLS_PC], ot[h:M, oc : oc + COLS_PC]
                )
    nc.compile()
    return nc


def _toeplitz(weight, np_dt):
    t = np.zeros((128, KW * MT), dtype=np.float32)
    idx = np.arange(MT)
    for kx in range(KW):
        for ky in range(KH):
            t[idx + ky, kx * MT + idx] = weight[ky, kx]
    return np.ascontiguousarray(t.astype(np_dt))


def kernel(x, weight):
    global LAST_EXEC_NS
    mode = MODE
    dt_b, np_dt = _DT[mode]
    if mode not in _compiled:
        _compiled[mode] = _build(mode)
    nc = _compiled[mode]

    xf = np.asarray(x, np.float32)
    wf = np.asarray(weight, np.float32)
    tmats = _toeplitz(wf, np_dt)
    xc = xf.astype(np_dt) if np_dt is not np.float32 else xf

    # padded canvas so every core's slab is [IN_ROWS, IN_COLS]
    xpad = np.zeros((ROW_BAND[-1] + IN_ROWS, COL_BAND[-1] + IN_COLS), dtype=xc.dtype)
    xpad[:H, :W] = xc
    in_maps = []
    for c in range(NCORES):
        r0, c0 = ROW_BAND[c // GC], COL_BAND[c % GC]
        in_maps.append(
            {
                "x": np.ascontiguousarray(xpad[r0 : r0 + IN_ROWS, c0 : c0 + IN_COLS]),
                "tmats": tmats,
            }
        )
    res = run_bass_kernel_spmd(
        nc, in_maps, core_ids=list(range(NCORES)), trace=TRACE
    )
    LAST_EXEC_NS = res.exec_time_ns

    out = np.empty((OH, OW), np.float32)
    for c in range(NCORES):
        r0, c0 = ROW_BAND[c // GC], COL_BAND[c % GC]
        od = res.results[c]["out"]  # [128, 17*1023]
        # od[p, s*1023 + cc] = out_core[122*s + p, cc]
        rows = (
            od.reshape(128, NSTRIPES, COLS_PC)
            .swapaxes(0, 1)
            .reshape(NSTRIPES * 128, COLS_PC)
        )
        # stripe s contributes rows 122s..122s+121 -> take p<122 of each stripe
        rows = rows.reshape(NSTRIPES, 128, COLS_PC)[:, :MT, :].reshape(-1, COLS_PC)
        out[r0 : r0 + ROWS_PC, c0 : c0 + COLS_PC] = rows[:ROWS_PC].astype(np.float32)
    return out
